# revision 14
# baseline (speedup 1.0000x reference)
"""3-layer GAT on 8 trn2 NeuronCores.

Strategy (graph/data parallel per sharding hint):
  - Nodes are assigned to 8 cores x 49 blocks x 128 slots (degree-balanced
    LPT bin packing) -> permuted node order; "table row" = block*128 + slot.
  - Per layer: each core transforms its own node shard with
    rhs = [W | W@as | W@ad] (alpha terms folded into the matmul), writes a
    table shard [6272, F+2H(padded)], AllGather -> full table on every core.
  - Aggregation: per dst-block of 128 nodes, edges (dst-sorted) are packed
    into 128-edge tiles; a dma_gather fetches table rows for the tile's
    sources; a one-hot "scatter matrix" matmul accumulates both the
    s_e-weighted feature sum and the softmax denominator into PSUM.
    (Softmax max-shift is skipped: logits are O(1) so exp is safe, and the
    result is mathematically identical.)
  - int16 gather indices: table split into lo rows [0,32768) and hi rows
    [17408,50176); per-block edges are balanced between the (overlapping)
    windows so each side fits 9 tiles of 128.
  - Layer 2 output is column-summed per core (masked for pad slots); the
    final mean + linear head run on host.
"""

import os
import numpy as np

# ---------------- problem constants (must match reference) ----------------
N = 50000
E = 800000
IN_C = 128
HID = 64
HEADS = 4
OUT_C = 64
F1 = HEADS * HID  # 256

# ---------------- sharding geometry ----------------
NCORES = 8
NB = 49           # dst blocks per core
BS = 128          # dst slots per block
NPC = NB * BS     # 6272 nodes per core
RTOT = NCORES * NPC  # 50176 table rows
TL = 9            # tiles per kind (lo/hi)
KE = TL * 128     # 1152 edge slots per (block, kind)
LO_LIM = 32768    # lo window rows [0, LO_LIM)
HI_BASE = 17408   # hi window rows [HI_BASE, HI_BASE+32768)
NKCOLS = KE // 16  # 72 idx columns per (block, kind)

USE_BF16 = os.environ.get("GAT_BF16", "0") == "1"

if USE_BF16:
    import ml_dtypes
    TB_NP = ml_dtypes.bfloat16
    EL01 = 384     # table elems/row layer0/1 (256 h + 4 as + 4 ad + pad)
    EL2 = 128      # table elems/row layer2 (64 h + 1 as + 1 ad + pad)
else:
    TB_NP = np.float32
    EL01 = 320
    EL2 = 128


# ---------------- host preprocessing ----------------

def preprocess(edge_index):
    """Node->(core,block,slot) assignment and per-core edge tile arrays.

    Returns dict with:
      row:   [N] table row of each node
      xperm: [RTOT] node id occupying each table row (-1 for pad slots)
      idx16: [NCORES,128,NB*2*NKCOLS] int16 wrapped gather indices
      dstc:  [NCORES,128,NB*2*TL] f32 dst_local per edge slot (col layout, -1 pad)
      dstr:  [NCORES,128,KE] f32 dst_local (row layout; partition=block*2+kind)
      maskc: [NCORES,128,NB] f32 1.0 for real-node slots
    """
    import heapq

    src = np.concatenate([np.asarray(edge_index[0]), np.arange(N, dtype=np.int64)])
    dst = np.concatenate([np.asarray(edge_index[1]), np.arange(N, dtype=np.int64)])
    deg = np.bincount(dst, minlength=N)

    nblocks = NCORES * NB
    order = np.argsort(-deg, kind="stable")
    heap = [(0, b) for b in range(nblocks)]
    heapq.heapify(heap)
    slots_used = np.zeros(nblocks, np.int64)
    node_block = np.empty(N, np.int64)
    node_slot = np.empty(N, np.int64)
    for n in order:
        popped = []
        while True:
            load, b = heapq.heappop(heap)
            if slots_used[b] < BS:
                break
            popped.append((load, b))
        node_block[n] = b
        node_slot[n] = slots_used[b]
        slots_used[b] += 1
        heapq.heappush(heap, (load + int(deg[n]), b))
        # blocks that were full stay out of the heap

    row = node_block * BS + node_slot  # table row per node

    xperm = np.full(RTOT, -1, np.int64)
    xperm[row] = np.arange(N)

    erow = row[src]          # gather row per edge
    eblk = node_block[dst]   # destination block per edge
    eslot = node_slot[dst]   # dst_local per edge

    idx16 = np.zeros((NCORES, 128, NB * 2 * NKCOLS), np.int16)
    dstc = np.full((NCORES, 128, NB * 2 * TL), -1.0, np.float32)
    dstr = np.zeros((NCORES, 128, KE), np.float32)
    maskc = np.zeros((NCORES, 128, NB), np.float32)

    order_e = np.argsort(eblk, kind="stable")
    bounds = np.searchsorted(eblk[order_e], np.arange(nblocks + 1))

    for b in range(nblocks):
        c, bl = divmod(b, NB)
        es = order_e[bounds[b]:bounds[b + 1]]
        r_ = erow[es]
        dl = eslot[es]
        lo_f = r_ < HI_BASE
        hi_f = r_ >= LO_LIM
        flex = ~lo_f & ~hi_f
        n_lo = int(lo_f.sum())
        n_hi = int(hi_f.sum())
        n_fx = int(flex.sum())
        tot = n_lo + n_hi + n_fx
        assert tot <= 2 * KE, f"block {b} has {tot} edges > {2*KE}"
        # send flex edges to lo until lo reaches ceil(tot/2) (capped at KE)
        add_lo = min(n_fx, max(0, min(KE, (tot + 1) // 2) - n_lo))
        if n_hi + (n_fx - add_lo) > KE:
            add_lo = n_fx - (KE - n_hi)
        assert 0 <= add_lo <= n_fx
        fx_idx = np.nonzero(flex)[0]
        sel_lo = np.zeros(len(es), bool)
        sel_lo[lo_f] = True
        sel_lo[fx_idx[:add_lo]] = True
        sel_hi = ~sel_lo
        assert sel_lo.sum() <= KE and sel_hi.sum() <= KE, (
            b, sel_lo.sum(), sel_hi.sum())

        for kind, sel, base in ((0, sel_lo, 0), (1, sel_hi, HI_BASE)):
            rr = r_[sel]
            dd = dl[sel]
            o = np.argsort(rr, kind="stable")  # DMA locality
            rr = rr[o]
            dd = dd[o]
            k = len(rr)
            rel = np.zeros(KE, np.int64)
            rel[:k] = rr - base
            dloc = np.full(KE, -1.0, np.float32)
            dloc[:k] = dd.astype(np.float32)
            assert rel.min() >= 0 and rel.max() < 32768
            # wrapped idx: index i -> [i % 16, i // 16]
            w = rel.reshape(NKCOLS, 16).T.astype(np.int16)  # [16, NKCOLS]
            cbase = (bl * 2 + kind) * NKCOLS
            idx16[c, :, cbase:cbase + NKCOLS] = np.tile(w, (8, 1))
            # col layout: col bl*2*TL + kind*TL + t, partition p = edge t*128+p
            tcol = bl * 2 * TL + kind * TL
            dstc[c, :, tcol:tcol + TL] = dloc.reshape(TL, 128).T
            # row layout: partition bl*2+kind
            dstr[c, bl * 2 + kind, :] = dloc

        # mask of real slots
        used = slots_used[b]
        maskc[c, :used, bl] = 1.0

    return dict(row=row, xperm=xperm, idx16=idx16, dstc=dstc, dstr=dstr,
                maskc=maskc, deg=deg, node_block=node_block,
                node_slot=node_slot)


def host_weights(inputs):
    """Extended weight matrices with folded attention vectors."""
    def ext(W, a_s, a_d, heads):
        # Was[k, h] = sum_c W[k, h*HID+c] * a_s[h, c]
        Wh = W.reshape(W.shape[0], heads, HID)
        Was = np.einsum("khc,hc->kh", Wh, a_s)
        Wad = np.einsum("khc,hc->kh", Wh, a_d)
        return np.concatenate([W, Was, Wad], axis=1).astype(np.float32)

    W0e = ext(np.asarray(inputs["W0"], np.float32),
              np.asarray(inputs["a0s"], np.float32),
              np.asarray(inputs["a0d"], np.float32), HEADS)      # [128, 264]
    W1e = ext(np.asarray(inputs["W1"], np.float32),
              np.asarray(inputs["a1s"], np.float32),
              np.asarray(inputs["a1d"], np.float32), HEADS)      # [256, 264]
    W2e = ext(np.asarray(inputs["W2"], np.float32),
              np.asarray(inputs["a2s"], np.float32),
              np.asarray(inputs["a2d"], np.float32), 1)          # [256, 66]
    return W0e, W1e, W2e


def build_core_inputs(inputs, pp):
    """Per-core in_maps for run_bass_kernel_spmd."""
    x = np.asarray(inputs["x"], np.float32)
    W0e, W1e, W2e = host_weights(inputs)
    b0 = np.asarray(inputs["b0"], np.float32)
    b1 = np.asarray(inputs["b1"], np.float32)
    b2 = np.asarray(inputs["b2"], np.float32)

    iota_row = np.tile(np.arange(128, dtype=np.float32), (128, 1))
    iota_col = np.arange(128, dtype=np.float32).reshape(128, 1)
    ones1 = np.ones((1, 128), np.float32)
    ident = np.eye(128, dtype=np.float32)

    consts = dict(
        w0e=W0e,                                    # [128, 264]
        w1e=W1e.reshape(2, 128, F1 + 2 * HEADS),    # [2, 128, 264]
        w2e=W2e.reshape(2, 128, HID + 2),           # [2, 128, 66]
        b0r=np.tile(b0, (128, 1)).astype(np.float32),
        b1r=np.tile(b1, (128, 1)).astype(np.float32),
        b2r=np.tile(b2, (128, 1)).astype(np.float32),
        iota_row=iota_row, iota_col=iota_col, ones1=ones1, ident=ident,
    )

    in_maps = []
    for c in range(NCORES):
        # xTb[b] = x[nodes of (c,b)].T : [128 feats, 128 slots]
        xtb = np.zeros((NB, IN_C, BS), np.float32)
        rows = np.arange(c * NPC, (c + 1) * NPC)
        nodes = pp["xperm"][rows].reshape(NB, BS)
        for b in range(NB):
            nb = nodes[b]
            valid = nb >= 0
            if valid.any():
                xtb[b][:, valid] = x[nb[valid]].T
        m = dict(
            xtb=xtb,
            idx16=pp["idx16"][c],
            dstc=pp["dstc"][c],
            dstr=pp["dstr"][c],
            maskc=pp["maskc"][c],
            **consts,
        )
        in_maps.append(m)
    return in_maps


# ---------------- numpy emulation of the device data path ----------------

def _emulate_layer(tables_in, pp, We, brep, heads, F_out, relu, el):
    """tables_in: hT equivalent — full node-major feature mat [RTOT, F_in].
    Returns (out [RTOT, F_out] node-major post-activation, table [RTOT, el])."""
    Fi = We.shape[0]
    Fo = F_out * 1
    # transform (all rows; pad rows produce garbage but are never gathered)
    tb = tables_in @ We  # [RTOT, Fo + 2*heads]
    table = np.zeros((RTOT, el), TB_NP)
    table[:, :Fo + 2 * heads] = tb.astype(TB_NP)
    ad_all = tb[:, Fo + heads:Fo + 2 * heads]  # [RTOT, heads]

    out = np.zeros((RTOT, Fo), np.float32)
    for c in range(NCORES):
        for bl in range(NB):
            rbase = c * NPC + bl * BS
            agg = np.zeros((BS, Fo), np.float32)
            den = np.zeros((BS, heads), np.float32)
            for kind in range(2):
                base = 0 if kind == 0 else HI_BASE
                cbase = (bl * 2 + kind) * NKCOLS
                w = pp["idx16"][c][:16, cbase:cbase + NKCOLS]
                rel = w.T.reshape(-1).astype(np.int64)  # unwrap
                rows = rel + base
                g = np.asarray(table[rows], np.float32)  # [KE, el]
                dl = pp["dstr"][c][bl * 2 + kind].astype(np.int64)  # -1 pads
                valid = dl >= 0
                a_s = g[:, Fo:Fo + heads]
                a_d = np.where(valid[:, None], ad_all[rbase + dl], 0.0)
                z = a_s + a_d
                s = np.exp(np.maximum(z, 0.2 * z)).astype(np.float32)
                hsc = (g[:, :Fo].reshape(KE, heads, HID)
                       * s[:, :, None]).astype(TB_NP).astype(np.float32)
                hsc = hsc.reshape(KE, Fo)
                np.add.at(agg, dl[valid], hsc[valid])
                np.add.at(den, dl[valid], s[valid])
            o = agg.reshape(BS, heads, HID) / (den + 1e-16)[:, :, None]
            o = o.reshape(BS, Fo) + brep[0]
            if relu:
                o = np.maximum(o, 0.0)
            out[rbase:rbase + BS] = o
    return out


def emulate(inputs, pp=None):
    """Full numpy emulation; returns [1, OUT_C]."""
    if pp is None:
        pp = preprocess(np.asarray(inputs["edge_index"]))
    x = np.asarray(inputs["x"], np.float32)
    W0e, W1e, W2e = host_weights(inputs)
    h = np.zeros((RTOT, IN_C), np.float32)
    valid = pp["xperm"] >= 0
    h[valid] = x[pp["xperm"][valid]]

    b0r = np.tile(np.asarray(inputs["b0"], np.float32), (1, 1))
    b1r = np.tile(np.asarray(inputs["b1"], np.float32), (1, 1))
    b2r = np.tile(np.asarray(inputs["b2"], np.float32), (1, 1))

    h0 = _emulate_layer(h, pp, W0e, b0r, HEADS, F1, True, EL01)
    h1 = _emulate_layer(h0, pp, W1e, b1r, HEADS, F1, True, EL01)
    h2 = _emulate_layer(h1, pp, W2e, b2r, 1, HID, False, EL2)

    g = h2[valid].sum(axis=0, keepdims=True) / N
    return (g @ np.asarray(inputs["hw"], np.float32)
            + np.asarray(inputs["hb"], np.float32)).astype(np.float32)


# ---------------- device kernel ----------------

_BUILT = None


def build_kernel(upto=99):
    import concourse.bacc as bacc
    import concourse.bass as bass
    import concourse.mybir as mybir
    import concourse.tile as tile
    from concourse import library_config

    f32 = mybir.dt.float32
    tb_dt = mybir.dt.bfloat16 if USE_BF16 else mybir.dt.float32
    i16 = mybir.dt.int16
    Alu = mybir.AluOpType
    Act = mybir.ActivationFunctionType

    nc = bacc.Bacc("TRN2", target_bir_lowering=False, debug=False,
                   num_devices=NCORES)

    # ---- I/O ----
    xtb_d = nc.dram_tensor("xtb", [NB, IN_C, BS], f32, kind="ExternalInput")
    idx16_d = nc.dram_tensor("idx16", [128, NB * 2 * NKCOLS], i16,
                             kind="ExternalInput")
    dstc_d = nc.dram_tensor("dstc", [128, NB * 2 * TL], f32,
                            kind="ExternalInput")
    dstr_d = nc.dram_tensor("dstr", [128, KE], f32, kind="ExternalInput")
    maskc_d = nc.dram_tensor("maskc", [128, NB], f32, kind="ExternalInput")
    w0e_d = nc.dram_tensor("w0e", [IN_C, F1 + 2 * HEADS], f32,
                           kind="ExternalInput")
    w1e_d = nc.dram_tensor("w1e", [2, 128, F1 + 2 * HEADS], f32,
                           kind="ExternalInput")
    w2e_d = nc.dram_tensor("w2e", [2, 128, HID + 2], f32,
                           kind="ExternalInput")
    b0r_d = nc.dram_tensor("b0r", [128, F1], f32, kind="ExternalInput")
    b1r_d = nc.dram_tensor("b1r", [128, F1], f32, kind="ExternalInput")
    b2r_d = nc.dram_tensor("b2r", [128, HID], f32, kind="ExternalInput")
    iota_row_d = nc.dram_tensor("iota_row", [128, 128], f32,
                                kind="ExternalInput")
    iota_col_d = nc.dram_tensor("iota_col", [128, 1], f32,
                                kind="ExternalInput")
    ones1_d = nc.dram_tensor("ones1", [1, 128], f32, kind="ExternalInput")
    ident_d = nc.dram_tensor("ident", [128, 128], f32, kind="ExternalInput")
    out_d = nc.dram_tensor("out_part", [1, OUT_C], f32, kind="ExternalOutput")
    debug = os.environ.get("GAT_DEBUG", "0") == "1"
    if debug:
        dmp_tb = nc.dram_tensor("dmp_tb", [NPC, EL01], f32,
                                kind="ExternalOutput")
        dmp_h = nc.dram_tensor("dmp_h", [NPC, F1], f32, kind="ExternalOutput")
        dmp_den = nc.dram_tensor("dmp_den", [NPC, HEADS], f32,
                                 kind="ExternalOutput")
        dmp_tmp = nc.dram_tensor("dmp_tmp", [128, TL, F1], f32,
                                 kind="ExternalOutput")
        dmp_agg = nc.dram_tensor("dmp_agg", [128, F1 + HEADS], f32,
                                 kind="ExternalOutput")
        dmp_g = nc.dram_tensor("dmp_g", [128, TL, EL01], f32,
                               kind="ExternalOutput")
        dmp_s = nc.dram_tensor("dmp_s", [128, TL * HEADS], f32,
                               kind="ExternalOutput")

    # internal DRAM
    shard01 = nc.dram_tensor("shard01", [NPC, EL01], tb_dt)
    table01 = nc.dram_tensor("table01", [RTOT, EL01], tb_dt)
    shard2 = nc.dram_tensor("shard2", [NPC, EL2], tb_dt)
    table2 = nc.dram_tensor("table2", [RTOT, EL2], tb_dt)

    rg = [list(range(NCORES))]

    with tile.TileContext(nc) as tc:
        with (
            tc.tile_pool(name="const", bufs=1) as cpool,
            tc.tile_pool(name="big", bufs=1) as bigpool,
            tc.tile_pool(name="work", bufs=3) as wpool,
            tc.tile_pool(name="gather", bufs=3) as gpool,
            tc.tile_pool(name="small", bufs=4) as spool,
            tc.tile_pool(name="psum", bufs=1, space="PSUM") as ppool,
        ):
            # ---- load constants ----
            def load_const(tag, dram, shape, dtype=f32, view=None):
                t = cpool.tile(shape, dtype, tag=tag)
                nc.sync.dma_start(out=t[:], in_=view if view is not None
                                  else dram[:])
                return t

            w0e_s = load_const("w0e", w0e_d, [IN_C, F1 + 2 * HEADS])
            w1e_s = load_const("w1e", w1e_d, [128, 2, F1 + 2 * HEADS],
                               view=w1e_d[:].rearrange("c p j -> p c j"))
            w2e_s = load_const("w2e", w2e_d, [128, 2, HID + 2],
                               view=w2e_d[:].rearrange("c p j -> p c j"))
            b0r_s = load_const("b0r", b0r_d, [128, F1])
            b1r_s = load_const("b1r", b1r_d, [128, F1])
            b2r_s = load_const("b2r", b2r_d, [128, HID])
            iota_row_s = load_const("iota_row", iota_row_d, [128, 128])
            iota_col_s = load_const("iota_col", iota_col_d, [128, 1])
            ones1_s = load_const("ones1", ones1_d, [1, 128])
            ident_s = load_const("ident", ident_d, [128, 128])
            idx16_s = load_const("idx16", idx16_d,
                                 [128, NB * 2 * NKCOLS], i16)
            dstc_s = load_const("dstc", dstc_d, [128, NB * 2 * TL])
            maskc_s = load_const("maskc", maskc_d, [128, NB])

            nc.gpsimd.load_library(library_config.mlp)

            hT = bigpool.tile([128, 2, NPC], f32, tag="hT")

            def transform(layer):
                """Own-shard transform -> shard DRAM + ad_all SBUF."""
                heads = 1 if layer == 2 else HEADS
                Fo = HID if layer == 2 else F1
                ncols = Fo + 2 * heads
                el = EL2 if layer == 2 else EL01
                shard = shard2 if layer == 2 else shard01
                ad_all = spool.tile([128, NB * heads], f32, tag="ad_all")
                for b in range(NB):
                    ps = ppool.tile([128, 512], f32, tag="agg", space="PSUM")
                    if layer == 0:
                        xb = wpool.tile([IN_C, BS], f32, tag="xtb")
                        nc.sync.dma_start(out=xb[:], in_=xtb_d[b])
                        nc.tensor.matmul(out=ps[:, :ncols], lhsT=xb[:],
                                         rhs=w0e_s[:], start=True, stop=True)
                    else:
                        we = w1e_s if layer == 1 else w2e_s
                        for k2 in range(2):
                            nc.tensor.matmul(
                                out=ps[:, :ncols],
                                lhsT=hT[:, k2, b * BS:(b + 1) * BS],
                                rhs=we[:, k2, :],
                                start=(k2 == 0), stop=(k2 == 1))
                    tb = wpool.tile([128, el], tb_dt, tag="tbout")
                    nc.vector.tensor_copy(out=tb[:, :ncols],
                                          in_=ps[:, :ncols])
                    nc.vector.tensor_copy(
                        out=ad_all[:, b * heads:(b + 1) * heads],
                        in_=ps[:, Fo + heads:Fo + 2 * heads])
                    nc.sync.dma_start(out=shard[b * BS:(b + 1) * BS, :],
                                      in_=tb[:])
                    if debug and layer == 0:
                        nc.sync.dma_start(
                            out=dmp_tb[b * BS:(b + 1) * BS, :], in_=tb[:])
                return ad_all

            def allgather(layer):
                shard = shard2 if layer == 2 else shard01
                table = table2 if layer == 2 else table01
                nc.gpsimd.collective_compute(
                    "AllGather", mybir.AluOpType.bypass,
                    replica_groups=rg, ins=[shard[:].opt()],
                    outs=[table[:].opt()])

            def aggregate(layer, ad_all):
                sub = int(os.environ.get("GAT_AGG_SUB", "99"))
                heads = 1 if layer == 2 else HEADS
                Fo = HID if layer == 2 else F1
                el = EL2 if layer == 2 else EL01
                table = table2 if layer == 2 else table01
                brep = (b2r_s, b1r_s, b1r_s)[0] if False else (
                    b0r_s if layer == 0 else (b1r_s if layer == 1 else b2r_s))
                views = [table[0:LO_LIM, :], table[HI_BASE:HI_BASE + 32768, :]]
                if layer == 2:
                    psum_sum = ppool.tile([1, OUT_C], f32, tag="sum",
                                          space="PSUM")
                for b in range(NB):
                    pagg = ppool.tile([128, Fo], f32, tag="agg",
                                      space="PSUM")
                    pden = ppool.tile([128, heads], f32, tag="den_ps",
                                      space="PSUM")
                    for kind in range(2):
                        bk = b * 2 + kind
                        g = gpool.tile([128, TL, el], tb_dt, tag="g")
                        nc.gpsimd.dma_gather(
                            g[:], views[kind],
                            idx16_s[:, bk * NKCOLS:(bk + 1) * NKCOLS],
                            KE, KE, el, single_packet=False)
                        if sub < 2:
                            continue
                        # one-hot M [128e, TL*128d]
                        M = wpool.tile([128, KE], tb_dt, tag="M")
                        tcol = b * 2 * TL + kind * TL
                        nc.vector.tensor_tensor(
                            out=M[:].rearrange("p (t d) -> p t d", t=TL),
                            in0=dstc_s[:, tcol:tcol + TL].unsqueeze(-1)
                                .broadcast_to([128, TL, 128]),
                            in1=iota_row_s[:].unsqueeze(1)
                                .broadcast_to([128, TL, 128]),
                            op=Alu.is_equal)
                        if sub < 3:
                            continue
                        # M_T [128d, TL*128e] via replicated-row outer product
                        MT = wpool.tile([128, KE], f32, tag="MT")
                        dr = spool.tile([1, KE], f32, tag="dr")
                        nc.sync.dma_start(out=dr[:], in_=dstr_d[bk:bk + 1, :])
                        for o, wdt in ((0, 512), (512, 512), (1024, 128)):
                            pr = ppool.tile([128, 512], f32, tag="rep",
                                            space="PSUM")
                            nc.tensor.matmul(out=pr[:, :wdt],
                                             lhsT=ones1_s[:],
                                             rhs=dr[:, o:o + wdt],
                                             start=True, stop=True)
                            nc.vector.tensor_tensor(
                                out=MT[:, o:o + wdt], in0=pr[:, :wdt],
                                in1=iota_col_s[:]
                                    .broadcast_to([128, wdt]),
                                op=Alu.is_equal)
                        if sub < 4:
                            continue
                        # ad per edge via M_T @ ad_block
                        pad_ = ppool.tile([128, TL * heads], f32, tag="adp",
                                          space="PSUM")
                        for t in range(TL):
                            nc.tensor.matmul(
                                out=pad_[:, t * heads:(t + 1) * heads],
                                lhsT=MT[:, t * 128:(t + 1) * 128],
                                rhs=ad_all[:, b * heads:(b + 1) * heads],
                                start=True, stop=True)
                        if sub < 5:
                            continue
                        # z = as + ad ; s = exp(max(z, 0.2 z))
                        z = spool.tile([128, TL * heads], f32, tag="z")
                        nc.vector.tensor_tensor(
                            out=z[:].rearrange("p (t h) -> p t h", t=TL),
                            in0=g[:, :, Fo:Fo + heads],
                            in1=pad_[:].rearrange("p (t h) -> p t h", t=TL),
                            op=Alu.add)
                        z2 = spool.tile([128, TL * heads], f32, tag="z2")
                        nc.vector.tensor_scalar(out=z2[:], in0=z[:],
                                                scalar1=0.2, scalar2=None,
                                                op0=Alu.mult)
                        zm = spool.tile([128, TL * heads], f32, tag="zm")
                        nc.vector.tensor_tensor(out=zm[:], in0=z[:],
                                                in1=z2[:], op=Alu.max)
                        s_t = spool.tile([128, TL * heads], tb_dt, tag="s")
                        nc.scalar.activation(s_t[:], zm[:], Act.Exp)
                        if sub < 6:
                            continue
                        # tmp = g[:, :, :Fo] * s (broadcast over HID),
                        # one 3D op per head (4D broadcast APs miscompute)
                        tmp = wpool.tile([128, TL, Fo], tb_dt, tag="tmp")
                        sv = s_t[:].rearrange("p (t h) -> p t h", t=TL)
                        for hh in range(heads):
                            nc.vector.tensor_tensor(
                                out=tmp[:, :, hh * HID:(hh + 1) * HID],
                                in0=g[:, :, hh * HID:(hh + 1) * HID],
                                in1=sv[:, :, hh:hh + 1]
                                    .broadcast_to([128, TL, HID]),
                                op=Alu.mult)
                        if debug and layer == 0 and b == 0 and kind == 0:
                            nc.sync.dma_start(out=dmp_tmp[:], in_=tmp[:])
                            nc.sync.dma_start(out=dmp_g[:], in_=g[:])
                            nc.sync.dma_start(out=dmp_s[:], in_=s_t[:])
                        if sub < 7:
                            continue
                        # accumulate
                        for t in range(TL):
                            first = (kind == 0 and t == 0)
                            last = (kind == 1 and t == TL - 1)
                            nc.tensor.matmul(
                                out=pagg[:],
                                lhsT=M[:, t * 128:(t + 1) * 128],
                                rhs=tmp[:, t, :],
                                start=first, stop=last)
                            nc.tensor.matmul(
                                out=pden[:],
                                lhsT=M[:, t * 128:(t + 1) * 128],
                                rhs=s_t[:, t * heads:(t + 1) * heads],
                                start=first, stop=last)
                    if sub < 8:
                        continue
                    # epilogue
                    if debug and layer == 0 and b == 0:
                        aggc = wpool.tile([128, F1 + HEADS], f32, tag="aggc")
                        nc.vector.tensor_copy(out=aggc[:, :F1], in_=pagg[:])
                        nc.vector.tensor_copy(out=aggc[:, F1:], in_=pden[:])
                        nc.sync.dma_start(out=dmp_agg[:], in_=aggc[:])
                    den = spool.tile([128, heads], f32, tag="den")
                    nc.vector.tensor_scalar(out=den[:],
                                            in0=pden[:],
                                            scalar1=1e-16, scalar2=None,
                                            op0=Alu.add)
                    rec = spool.tile([128, heads], f32, tag="rec")
                    nc.vector.reciprocal(out=rec[:], in_=den[:])
                    if debug and layer == 0:
                        nc.sync.dma_start(
                            out=dmp_den[b * BS:(b + 1) * BS, :], in_=den[:])
                    o1 = wpool.tile([128, Fo], f32, tag="o1")
                    nc.vector.tensor_tensor(
                        out=o1[:].rearrange("p (h f) -> p h f", h=heads),
                        in0=pagg[:].rearrange("p (h f) -> p h f",
                                              h=heads),
                        in1=rec[:].unsqueeze(-1)
                            .broadcast_to([128, heads, HID]),
                        op=Alu.mult)
                    o2 = wpool.tile([128, Fo], f32, tag="o2")
                    nc.vector.tensor_tensor(out=o2[:], in0=o1[:],
                                            in1=brep[:, :Fo], op=Alu.add)
                    if layer == 2:
                        nc.tensor.matmul(out=psum_sum[:],
                                         lhsT=maskc_s[:, b:b + 1],
                                         rhs=o2[:], start=(b == 0),
                                         stop=(b == NB - 1))
                    else:
                        o3 = wpool.tile([128, Fo], f32, tag="o3")
                        nc.scalar.activation(o3[:], o2[:], Act.Relu)
                        if debug and layer == 0:
                            nc.sync.dma_start(
                                out=dmp_h[b * BS:(b + 1) * BS, :], in_=o3[:])
                        for k2 in range(2):
                            pt = ppool.tile([128, 128], f32, tag="tp",
                                            space="PSUM")
                            nc.tensor.transpose(
                                pt[:], o3[:, k2 * 128:(k2 + 1) * 128],
                                ident_s[:])
                            nc.vector.tensor_copy(
                                out=hT[:, k2, b * BS:(b + 1) * BS],
                                in_=pt[:])
                if layer == 2:
                    osb = spool.tile([1, OUT_C], f32, tag="osb")
                    nc.vector.tensor_copy(out=osb[:], in_=psum_sum[:])
                    nc.sync.dma_start(out=out_d[:], in_=osb[:])

            stage = 0
            for layer in range(3):
                if stage >= upto:
                    break
                ad_all = transform(layer)
                stage += 1
                if stage >= upto:
                    break
                allgather(layer)
                stage += 1
                if stage >= upto:
                    break
                aggregate(layer, ad_all)
                stage += 1

    nc.compile()
    return nc


def _get_built():
    global _BUILT
    if _BUILT is None:
        _BUILT = build_kernel(upto=int(os.environ.get("GAT_UPTO", "99")))
    return _BUILT


def kernel(**inputs) -> np.ndarray:
    from concourse.bass_utils import run_bass_kernel_spmd

    pp = preprocess(np.asarray(inputs["edge_index"]))
    in_maps = build_core_inputs(inputs, pp)
    nc = _get_built()
    res = run_bass_kernel_spmd(nc, in_maps, core_ids=list(range(NCORES)))
    parts = np.stack([r["out_part"][0] for r in res.results])  # [8, 64]
    g = parts.sum(axis=0, keepdims=True) / N
    out = (g @ np.asarray(inputs["hw"], np.float32)
           + np.asarray(inputs["hb"], np.float32)).astype(np.float32)
    return out


# revision 15
# speedup vs baseline: 1.5298x; 1.5298x over previous
"""3-layer GAT on 8 trn2 NeuronCores.

Strategy (graph/data parallel per sharding hint):
  - Nodes are assigned to 8 cores x 49 blocks x 128 slots (degree-balanced
    LPT bin packing) -> permuted node order; "table row" = block*128 + slot.
  - Per layer: each core transforms its own node shard with
    rhs = [W | W@as | W@ad] (alpha terms folded into the matmul), writes a
    table shard [6272, F+2H(padded)], AllGather -> full table on every core.
  - Aggregation: per dst-block of 128 nodes, edges (dst-sorted) are packed
    into 128-edge tiles; a dma_gather fetches table rows for the tile's
    sources; a one-hot "scatter matrix" matmul accumulates both the
    s_e-weighted feature sum and the softmax denominator into PSUM.
    (Softmax max-shift is skipped: logits are O(1) so exp is safe, and the
    result is mathematically identical.)
  - int16 gather indices: table split into lo rows [0,32768) and hi rows
    [17408,50176); per-block edges are balanced between the (overlapping)
    windows so each side fits 9 tiles of 128.
  - Layer 2 output is column-summed per core (masked for pad slots); the
    final mean + linear head run on host.
"""

import os
import numpy as np

# ---------------- problem constants (must match reference) ----------------
N = 50000
E = 800000
IN_C = 128
HID = 64
HEADS = 4
OUT_C = 64
F1 = HEADS * HID  # 256

# ---------------- sharding geometry ----------------
NCORES = 8
NB = 49           # dst blocks per core
BS = 128          # dst slots per block
NPC = NB * BS     # 6272 nodes per core
RTOT = NCORES * NPC  # 50176 table rows
TL = 9            # tiles per kind (lo/hi)
KE = TL * 128     # 1152 edge slots per (block, kind)
LO_LIM = 32768    # lo window rows [0, LO_LIM)
HI_BASE = 17408   # hi window rows [HI_BASE, HI_BASE+32768)
NKCOLS = KE // 16  # 72 idx columns per (block, kind)

USE_BF16 = os.environ.get("GAT_BF16", "0") == "1"

if USE_BF16:
    import ml_dtypes
    TB_NP = ml_dtypes.bfloat16
    EL01 = 384     # table elems/row layer0/1 (256 h + 4 as + 4 ad + pad)
    EL2 = 128      # table elems/row layer2 (64 h + 1 as + 1 ad + pad)
else:
    TB_NP = np.float32
    EL01 = 320
    EL2 = 128


# ---------------- host preprocessing ----------------

def preprocess(edge_index):
    """Node->(core,block,slot) assignment and per-core edge tile arrays.

    Returns dict with:
      row:   [N] table row of each node
      xperm: [RTOT] node id occupying each table row (-1 for pad slots)
      idx16: [NCORES,128,NB*2*NKCOLS] int16 wrapped gather indices
      dstc:  [NCORES,128,NB*2*TL] f32 dst_local per edge slot (col layout, -1 pad)
      dstr:  [NCORES,128,KE] f32 dst_local (row layout; partition=block*2+kind)
      maskc: [NCORES,128,NB] f32 1.0 for real-node slots
    """
    import heapq

    src = np.concatenate([np.asarray(edge_index[0]), np.arange(N, dtype=np.int64)])
    dst = np.concatenate([np.asarray(edge_index[1]), np.arange(N, dtype=np.int64)])
    deg = np.bincount(dst, minlength=N)

    nblocks = NCORES * NB
    order = np.argsort(-deg, kind="stable")
    heap = [(0, b) for b in range(nblocks)]
    heapq.heapify(heap)
    slots_used = np.zeros(nblocks, np.int64)
    node_block = np.empty(N, np.int64)
    node_slot = np.empty(N, np.int64)
    for n in order:
        popped = []
        while True:
            load, b = heapq.heappop(heap)
            if slots_used[b] < BS:
                break
            popped.append((load, b))
        node_block[n] = b
        node_slot[n] = slots_used[b]
        slots_used[b] += 1
        heapq.heappush(heap, (load + int(deg[n]), b))
        # blocks that were full stay out of the heap

    row = node_block * BS + node_slot  # table row per node

    xperm = np.full(RTOT, -1, np.int64)
    xperm[row] = np.arange(N)

    erow = row[src]          # gather row per edge
    eblk = node_block[dst]   # destination block per edge
    eslot = node_slot[dst]   # dst_local per edge

    idx16 = np.zeros((NCORES, 128, NB * 2 * NKCOLS), np.int16)
    dstc = np.full((NCORES, 128, NB * 2 * TL), -1.0, np.float32)
    dstr = np.zeros((NCORES, 128, KE), np.float32)
    maskc = np.zeros((NCORES, 128, NB), np.float32)

    order_e = np.argsort(eblk, kind="stable")
    bounds = np.searchsorted(eblk[order_e], np.arange(nblocks + 1))

    for b in range(nblocks):
        c, bl = divmod(b, NB)
        es = order_e[bounds[b]:bounds[b + 1]]
        r_ = erow[es]
        dl = eslot[es]
        lo_f = r_ < HI_BASE
        hi_f = r_ >= LO_LIM
        flex = ~lo_f & ~hi_f
        n_lo = int(lo_f.sum())
        n_hi = int(hi_f.sum())
        n_fx = int(flex.sum())
        tot = n_lo + n_hi + n_fx
        assert tot <= 2 * KE, f"block {b} has {tot} edges > {2*KE}"
        # send flex edges to lo until lo reaches ceil(tot/2) (capped at KE)
        add_lo = min(n_fx, max(0, min(KE, (tot + 1) // 2) - n_lo))
        if n_hi + (n_fx - add_lo) > KE:
            add_lo = n_fx - (KE - n_hi)
        assert 0 <= add_lo <= n_fx
        fx_idx = np.nonzero(flex)[0]
        sel_lo = np.zeros(len(es), bool)
        sel_lo[lo_f] = True
        sel_lo[fx_idx[:add_lo]] = True
        sel_hi = ~sel_lo
        assert sel_lo.sum() <= KE and sel_hi.sum() <= KE, (
            b, sel_lo.sum(), sel_hi.sum())

        for kind, sel, base in ((0, sel_lo, 0), (1, sel_hi, HI_BASE)):
            rr = r_[sel]
            dd = dl[sel]
            o = np.argsort(rr, kind="stable")  # DMA locality
            rr = rr[o]
            dd = dd[o]
            k = len(rr)
            rel = np.zeros(KE, np.int64)
            rel[:k] = rr - base
            dloc = np.full(KE, -1.0, np.float32)
            dloc[:k] = dd.astype(np.float32)
            assert rel.min() >= 0 and rel.max() < 32768
            # wrapped idx: index i -> [i % 16, i // 16]
            w = rel.reshape(NKCOLS, 16).T.astype(np.int16)  # [16, NKCOLS]
            cbase = (bl * 2 + kind) * NKCOLS
            idx16[c, :, cbase:cbase + NKCOLS] = np.tile(w, (8, 1))
            # col layout: col bl*2*TL + kind*TL + t, partition p = edge t*128+p
            tcol = bl * 2 * TL + kind * TL
            dstc[c, :, tcol:tcol + TL] = dloc.reshape(TL, 128).T
            # row layout: partition bl*2+kind
            dstr[c, bl * 2 + kind, :] = dloc

        # mask of real slots
        used = slots_used[b]
        maskc[c, :used, bl] = 1.0

    return dict(row=row, xperm=xperm, idx16=idx16, dstc=dstc, dstr=dstr,
                maskc=maskc, deg=deg, node_block=node_block,
                node_slot=node_slot)


def host_weights(inputs):
    """Extended weight matrices with folded attention vectors."""
    def ext(W, a_s, a_d, heads):
        # Was[k, h] = sum_c W[k, h*HID+c] * a_s[h, c]
        Wh = W.reshape(W.shape[0], heads, HID)
        Was = np.einsum("khc,hc->kh", Wh, a_s)
        Wad = np.einsum("khc,hc->kh", Wh, a_d)
        return np.concatenate([W, Was, Wad], axis=1).astype(np.float32)

    W0e = ext(np.asarray(inputs["W0"], np.float32),
              np.asarray(inputs["a0s"], np.float32),
              np.asarray(inputs["a0d"], np.float32), HEADS)      # [128, 264]
    W1e = ext(np.asarray(inputs["W1"], np.float32),
              np.asarray(inputs["a1s"], np.float32),
              np.asarray(inputs["a1d"], np.float32), HEADS)      # [256, 264]
    W2e = ext(np.asarray(inputs["W2"], np.float32),
              np.asarray(inputs["a2s"], np.float32),
              np.asarray(inputs["a2d"], np.float32), 1)          # [256, 66]
    return W0e, W1e, W2e


def build_core_inputs(inputs, pp):
    """Per-core in_maps for run_bass_kernel_spmd."""
    x = np.asarray(inputs["x"], np.float32)
    W0e, W1e, W2e = host_weights(inputs)
    b0 = np.asarray(inputs["b0"], np.float32)
    b1 = np.asarray(inputs["b1"], np.float32)
    b2 = np.asarray(inputs["b2"], np.float32)

    iota_row = np.tile(np.arange(128, dtype=np.float32), (128, 1))
    iota_col = np.arange(128, dtype=np.float32).reshape(128, 1)
    ones1 = np.ones((1, 128), np.float32)
    ident = np.eye(128, dtype=np.float32)

    consts = dict(
        w0e=W0e,                                    # [128, 264]
        w1e=W1e.reshape(2, 128, F1 + 2 * HEADS),    # [2, 128, 264]
        w2e=W2e.reshape(2, 128, HID + 2),           # [2, 128, 66]
        b0r=np.tile(b0, (128, 1)).astype(np.float32),
        b1r=np.tile(b1, (128, 1)).astype(np.float32),
        b2r=np.tile(b2, (128, 1)).astype(np.float32),
        iota_row=iota_row, iota_col=iota_col, ones1=ones1, ident=ident,
    )

    in_maps = []
    for c in range(NCORES):
        # xTb[b] = x[nodes of (c,b)].T : [128 feats, 128 slots]
        xtb = np.zeros((NB, IN_C, BS), np.float32)
        rows = np.arange(c * NPC, (c + 1) * NPC)
        nodes = pp["xperm"][rows].reshape(NB, BS)
        for b in range(NB):
            nb = nodes[b]
            valid = nb >= 0
            if valid.any():
                xtb[b][:, valid] = x[nb[valid]].T
        m = dict(
            xtb=xtb,
            idx16=pp["idx16"][c],
            dstc=pp["dstc"][c],
            dstr=pp["dstr"][c],
            maskc=pp["maskc"][c],
            **consts,
        )
        in_maps.append(m)
    return in_maps


# ---------------- numpy emulation of the device data path ----------------

def _emulate_layer(tables_in, pp, We, brep, heads, F_out, relu, el):
    """tables_in: hT equivalent — full node-major feature mat [RTOT, F_in].
    Returns (out [RTOT, F_out] node-major post-activation, table [RTOT, el])."""
    Fi = We.shape[0]
    Fo = F_out * 1
    # transform (all rows; pad rows produce garbage but are never gathered)
    tb = tables_in @ We  # [RTOT, Fo + 2*heads]
    table = np.zeros((RTOT, el), TB_NP)
    table[:, :Fo + 2 * heads] = tb.astype(TB_NP)
    ad_all = tb[:, Fo + heads:Fo + 2 * heads]  # [RTOT, heads]

    out = np.zeros((RTOT, Fo), np.float32)
    for c in range(NCORES):
        for bl in range(NB):
            rbase = c * NPC + bl * BS
            agg = np.zeros((BS, Fo), np.float32)
            den = np.zeros((BS, heads), np.float32)
            for kind in range(2):
                base = 0 if kind == 0 else HI_BASE
                cbase = (bl * 2 + kind) * NKCOLS
                w = pp["idx16"][c][:16, cbase:cbase + NKCOLS]
                rel = w.T.reshape(-1).astype(np.int64)  # unwrap
                rows = rel + base
                g = np.asarray(table[rows], np.float32)  # [KE, el]
                dl = pp["dstr"][c][bl * 2 + kind].astype(np.int64)  # -1 pads
                valid = dl >= 0
                a_s = g[:, Fo:Fo + heads]
                a_d = np.where(valid[:, None], ad_all[rbase + dl], 0.0)
                z = a_s + a_d
                s = np.exp(np.maximum(z, 0.2 * z)).astype(np.float32)
                hsc = (g[:, :Fo].reshape(KE, heads, HID)
                       * s[:, :, None]).astype(TB_NP).astype(np.float32)
                hsc = hsc.reshape(KE, Fo)
                np.add.at(agg, dl[valid], hsc[valid])
                np.add.at(den, dl[valid], s[valid])
            o = agg.reshape(BS, heads, HID) / (den + 1e-16)[:, :, None]
            o = o.reshape(BS, Fo) + brep[0]
            if relu:
                o = np.maximum(o, 0.0)
            out[rbase:rbase + BS] = o
    return out


def emulate(inputs, pp=None):
    """Full numpy emulation; returns [1, OUT_C]."""
    if pp is None:
        pp = preprocess(np.asarray(inputs["edge_index"]))
    x = np.asarray(inputs["x"], np.float32)
    W0e, W1e, W2e = host_weights(inputs)
    h = np.zeros((RTOT, IN_C), np.float32)
    valid = pp["xperm"] >= 0
    h[valid] = x[pp["xperm"][valid]]

    b0r = np.tile(np.asarray(inputs["b0"], np.float32), (1, 1))
    b1r = np.tile(np.asarray(inputs["b1"], np.float32), (1, 1))
    b2r = np.tile(np.asarray(inputs["b2"], np.float32), (1, 1))

    h0 = _emulate_layer(h, pp, W0e, b0r, HEADS, F1, True, EL01)
    h1 = _emulate_layer(h0, pp, W1e, b1r, HEADS, F1, True, EL01)
    h2 = _emulate_layer(h1, pp, W2e, b2r, 1, HID, False, EL2)

    g = h2[valid].sum(axis=0, keepdims=True) / N
    return (g @ np.asarray(inputs["hw"], np.float32)
            + np.asarray(inputs["hb"], np.float32)).astype(np.float32)


# ---------------- device kernel ----------------

_BUILT = None


def build_kernel(upto=99):
    import concourse.bacc as bacc
    import concourse.bass as bass
    import concourse.mybir as mybir
    import concourse.tile as tile
    from concourse import library_config

    f32 = mybir.dt.float32
    tb_dt = mybir.dt.bfloat16 if USE_BF16 else mybir.dt.float32
    i16 = mybir.dt.int16
    Alu = mybir.AluOpType
    Act = mybir.ActivationFunctionType

    nc = bacc.Bacc("TRN2", target_bir_lowering=False, debug=False,
                   num_devices=NCORES)

    # ---- I/O ----
    xtb_d = nc.dram_tensor("xtb", [NB, IN_C, BS], f32, kind="ExternalInput")
    idx16_d = nc.dram_tensor("idx16", [128, NB * 2 * NKCOLS], i16,
                             kind="ExternalInput")
    dstc_d = nc.dram_tensor("dstc", [128, NB * 2 * TL], f32,
                            kind="ExternalInput")
    dstr_d = nc.dram_tensor("dstr", [128, KE], f32, kind="ExternalInput")
    maskc_d = nc.dram_tensor("maskc", [128, NB], f32, kind="ExternalInput")
    w0e_d = nc.dram_tensor("w0e", [IN_C, F1 + 2 * HEADS], f32,
                           kind="ExternalInput")
    w1e_d = nc.dram_tensor("w1e", [2, 128, F1 + 2 * HEADS], f32,
                           kind="ExternalInput")
    w2e_d = nc.dram_tensor("w2e", [2, 128, HID + 2], f32,
                           kind="ExternalInput")
    b0r_d = nc.dram_tensor("b0r", [128, F1], f32, kind="ExternalInput")
    b1r_d = nc.dram_tensor("b1r", [128, F1], f32, kind="ExternalInput")
    b2r_d = nc.dram_tensor("b2r", [128, HID], f32, kind="ExternalInput")
    iota_row_d = nc.dram_tensor("iota_row", [128, 128], f32,
                                kind="ExternalInput")
    iota_col_d = nc.dram_tensor("iota_col", [128, 1], f32,
                                kind="ExternalInput")
    ones1_d = nc.dram_tensor("ones1", [1, 128], f32, kind="ExternalInput")
    ident_d = nc.dram_tensor("ident", [128, 128], f32, kind="ExternalInput")
    out_d = nc.dram_tensor("out_part", [1, OUT_C], f32, kind="ExternalOutput")
    debug = os.environ.get("GAT_DEBUG", "0") == "1"
    if debug:
        dmp_tb = nc.dram_tensor("dmp_tb", [NPC, EL01], f32,
                                kind="ExternalOutput")
        dmp_h = nc.dram_tensor("dmp_h", [NPC, F1], f32, kind="ExternalOutput")
        dmp_den = nc.dram_tensor("dmp_den", [NPC, HEADS], f32,
                                 kind="ExternalOutput")
        dmp_tmp = nc.dram_tensor("dmp_tmp", [128, TL, F1], f32,
                                 kind="ExternalOutput")
        dmp_agg = nc.dram_tensor("dmp_agg", [128, F1 + HEADS], f32,
                                 kind="ExternalOutput")
        dmp_g = nc.dram_tensor("dmp_g", [128, TL, EL01], f32,
                               kind="ExternalOutput")
        dmp_s = nc.dram_tensor("dmp_s", [128, TL * HEADS], f32,
                               kind="ExternalOutput")

    # internal DRAM
    shard01 = nc.dram_tensor("shard01", [NPC, EL01], tb_dt)
    table01 = nc.dram_tensor("table01", [RTOT, EL01], tb_dt)
    shard2 = nc.dram_tensor("shard2", [NPC, EL2], tb_dt)
    table2 = nc.dram_tensor("table2", [RTOT, EL2], tb_dt)

    rg = [list(range(NCORES))]

    with tile.TileContext(nc) as tc:
        with (
            tc.tile_pool(name="const", bufs=1) as cpool,
            tc.tile_pool(name="big", bufs=1) as bigpool,
            tc.tile_pool(name="work", bufs=3) as wpool,
            tc.tile_pool(name="gather", bufs=3) as gpool,
            tc.tile_pool(name="small", bufs=4) as spool,
            tc.tile_pool(name="psum", bufs=2, space="PSUM") as ppool,
            tc.tile_pool(name="psum1", bufs=1, space="PSUM") as ppool1,
        ):
            # ---- load constants ----
            def load_const(tag, dram, shape, dtype=f32, view=None):
                t = cpool.tile(shape, dtype, tag=tag)
                nc.sync.dma_start(out=t[:], in_=view if view is not None
                                  else dram[:])
                return t

            w0e_s = load_const("w0e", w0e_d, [IN_C, F1 + 2 * HEADS])
            w1e_s = load_const("w1e", w1e_d, [128, 2, F1 + 2 * HEADS],
                               view=w1e_d[:].rearrange("c p j -> p c j"))
            w2e_s = load_const("w2e", w2e_d, [128, 2, HID + 2],
                               view=w2e_d[:].rearrange("c p j -> p c j"))
            b0r_s = load_const("b0r", b0r_d, [128, F1])
            b1r_s = load_const("b1r", b1r_d, [128, F1])
            b2r_s = load_const("b2r", b2r_d, [128, HID])
            iota_row_s = load_const("iota_row", iota_row_d, [128, 128])
            iota_col_s = load_const("iota_col", iota_col_d, [128, 1])
            ones1_s = load_const("ones1", ones1_d, [1, 128])
            ident_s = load_const("ident", ident_d, [128, 128])
            idx16_s = load_const("idx16", idx16_d,
                                 [128, NB * 2 * NKCOLS], i16)
            dstc_s = load_const("dstc", dstc_d, [128, NB * 2 * TL])
            maskc_s = load_const("maskc", maskc_d, [128, NB])

            nc.gpsimd.load_library(library_config.mlp)

            hT = bigpool.tile([128, 2, NPC], f32, tag="hT")

            def transform(layer):
                """Own-shard transform -> shard DRAM + ad_all SBUF."""
                heads = 1 if layer == 2 else HEADS
                Fo = HID if layer == 2 else F1
                ncols = Fo + 2 * heads
                el = EL2 if layer == 2 else EL01
                shard = shard2 if layer == 2 else shard01
                ad_all = spool.tile([128, NB * heads], f32, tag="ad_all")
                for b in range(NB):
                    ps = ppool.tile([128, 512], f32, tag="agg", space="PSUM")
                    if layer == 0:
                        xb = wpool.tile([IN_C, BS], f32, tag="xtb")
                        nc.sync.dma_start(out=xb[:], in_=xtb_d[b])
                        nc.tensor.matmul(out=ps[:, :ncols], lhsT=xb[:],
                                         rhs=w0e_s[:], start=True, stop=True)
                    else:
                        we = w1e_s if layer == 1 else w2e_s
                        for k2 in range(2):
                            nc.tensor.matmul(
                                out=ps[:, :ncols],
                                lhsT=hT[:, k2, b * BS:(b + 1) * BS],
                                rhs=we[:, k2, :],
                                start=(k2 == 0), stop=(k2 == 1))
                    tb = wpool.tile([128, el], tb_dt, tag="tbout")
                    nc.vector.tensor_copy(out=tb[:, :ncols],
                                          in_=ps[:, :ncols])
                    nc.vector.tensor_copy(
                        out=ad_all[:, b * heads:(b + 1) * heads],
                        in_=ps[:, Fo + heads:Fo + 2 * heads])
                    nc.sync.dma_start(out=shard[b * BS:(b + 1) * BS, :],
                                      in_=tb[:])
                    if debug and layer == 0:
                        nc.sync.dma_start(
                            out=dmp_tb[b * BS:(b + 1) * BS, :], in_=tb[:])
                return ad_all

            def allgather(layer):
                shard = shard2 if layer == 2 else shard01
                table = table2 if layer == 2 else table01
                nc.gpsimd.collective_compute(
                    "AllGather", mybir.AluOpType.bypass,
                    replica_groups=rg, ins=[shard[:].opt()],
                    outs=[table[:].opt()])

            def aggregate(layer, ad_all):
                sub = int(os.environ.get("GAT_AGG_SUB", "99"))
                heads = 1 if layer == 2 else HEADS
                Fo = HID if layer == 2 else F1
                el = EL2 if layer == 2 else EL01
                table = table2 if layer == 2 else table01
                brep = (b2r_s, b1r_s, b1r_s)[0] if False else (
                    b0r_s if layer == 0 else (b1r_s if layer == 1 else b2r_s))
                views = [table[0:LO_LIM, :], table[HI_BASE:HI_BASE + 32768, :]]
                if layer == 2:
                    psum_sum = ppool1.tile([1, OUT_C], f32, tag="sum",
                                          space="PSUM")
                for b in range(NB):
                    pagg = ppool.tile([128, Fo], f32, tag="agg",
                                      space="PSUM")
                    pden = ppool.tile([128, heads], f32, tag="den_ps",
                                      space="PSUM")
                    for kind in range(2):
                        bk = b * 2 + kind
                        g = gpool.tile([128, TL, el], tb_dt, tag="g")
                        nc.gpsimd.dma_gather(
                            g[:], views[kind],
                            idx16_s[:, bk * NKCOLS:(bk + 1) * NKCOLS],
                            KE, KE, el, single_packet=False)
                        if sub < 2:
                            continue
                        # one-hot M [128e, TL*128d]
                        M = wpool.tile([128, KE], tb_dt, tag="M")
                        tcol = b * 2 * TL + kind * TL
                        nc.vector.tensor_tensor(
                            out=M[:].rearrange("p (t d) -> p t d", t=TL),
                            in0=dstc_s[:, tcol:tcol + TL].unsqueeze(-1)
                                .broadcast_to([128, TL, 128]),
                            in1=iota_row_s[:].unsqueeze(1)
                                .broadcast_to([128, TL, 128]),
                            op=Alu.is_equal)
                        if sub < 3:
                            continue
                        # M_T [128d, TL*128e] via replicated-row outer product
                        MT = wpool.tile([128, KE], f32, tag="MT")
                        dr = spool.tile([1, KE], f32, tag="dr")
                        nc.sync.dma_start(out=dr[:], in_=dstr_d[bk:bk + 1, :])
                        for o, wdt in ((0, 512), (512, 512), (1024, 128)):
                            pr = ppool1.tile([128, 512], f32, tag="rep",
                                            space="PSUM")
                            nc.tensor.matmul(out=pr[:, :wdt],
                                             lhsT=ones1_s[:],
                                             rhs=dr[:, o:o + wdt],
                                             start=True, stop=True)
                            nc.vector.tensor_tensor(
                                out=MT[:, o:o + wdt], in0=pr[:, :wdt],
                                in1=iota_col_s[:]
                                    .broadcast_to([128, wdt]),
                                op=Alu.is_equal)
                        if sub < 4:
                            continue
                        # ad per edge via M_T @ ad_block
                        pad_ = ppool1.tile([128, TL * heads], f32, tag="adp",
                                          space="PSUM")
                        for t in range(TL):
                            nc.tensor.matmul(
                                out=pad_[:, t * heads:(t + 1) * heads],
                                lhsT=MT[:, t * 128:(t + 1) * 128],
                                rhs=ad_all[:, b * heads:(b + 1) * heads],
                                start=True, stop=True)
                        if sub < 5:
                            continue
                        # z = as + ad ; s = exp(max(z, 0.2 z))
                        z = spool.tile([128, TL * heads], f32, tag="z")
                        nc.vector.tensor_tensor(
                            out=z[:].rearrange("p (t h) -> p t h", t=TL),
                            in0=g[:, :, Fo:Fo + heads],
                            in1=pad_[:].rearrange("p (t h) -> p t h", t=TL),
                            op=Alu.add)
                        z2 = spool.tile([128, TL * heads], f32, tag="z2")
                        nc.vector.tensor_scalar(out=z2[:], in0=z[:],
                                                scalar1=0.2, scalar2=None,
                                                op0=Alu.mult)
                        zm = spool.tile([128, TL * heads], f32, tag="zm")
                        nc.vector.tensor_tensor(out=zm[:], in0=z[:],
                                                in1=z2[:], op=Alu.max)
                        s_t = spool.tile([128, TL * heads], tb_dt, tag="s")
                        nc.scalar.activation(s_t[:], zm[:], Act.Exp)
                        if sub < 6:
                            continue
                        # tmp = g[:, :, :Fo] * s (broadcast over HID),
                        # one 3D op per head (4D broadcast APs miscompute)
                        tmp = wpool.tile([128, TL, Fo], tb_dt, tag="tmp")
                        sv = s_t[:].rearrange("p (t h) -> p t h", t=TL)
                        for hh in range(heads):
                            nc.vector.tensor_tensor(
                                out=tmp[:, :, hh * HID:(hh + 1) * HID],
                                in0=g[:, :, hh * HID:(hh + 1) * HID],
                                in1=sv[:, :, hh:hh + 1]
                                    .broadcast_to([128, TL, HID]),
                                op=Alu.mult)
                        if debug and layer == 0 and b == 0 and kind == 0:
                            nc.sync.dma_start(out=dmp_tmp[:], in_=tmp[:])
                            nc.sync.dma_start(out=dmp_g[:], in_=g[:])
                            nc.sync.dma_start(out=dmp_s[:], in_=s_t[:])
                        if sub < 7:
                            continue
                        # accumulate
                        for t in range(TL):
                            first = (kind == 0 and t == 0)
                            last = (kind == 1 and t == TL - 1)
                            nc.tensor.matmul(
                                out=pagg[:],
                                lhsT=M[:, t * 128:(t + 1) * 128],
                                rhs=tmp[:, t, :],
                                start=first, stop=last)
                            nc.tensor.matmul(
                                out=pden[:],
                                lhsT=M[:, t * 128:(t + 1) * 128],
                                rhs=s_t[:, t * heads:(t + 1) * heads],
                                start=first, stop=last)
                    if sub < 8:
                        continue
                    # epilogue
                    if debug and layer == 0 and b == 0:
                        aggc = wpool.tile([128, F1 + HEADS], f32, tag="aggc")
                        nc.vector.tensor_copy(out=aggc[:, :F1], in_=pagg[:])
                        nc.vector.tensor_copy(out=aggc[:, F1:], in_=pden[:])
                        nc.sync.dma_start(out=dmp_agg[:], in_=aggc[:])
                    den = spool.tile([128, heads], f32, tag="den")
                    nc.vector.tensor_scalar(out=den[:],
                                            in0=pden[:],
                                            scalar1=1e-16, scalar2=None,
                                            op0=Alu.add)
                    rec = spool.tile([128, heads], f32, tag="rec")
                    nc.vector.reciprocal(out=rec[:], in_=den[:])
                    if debug and layer == 0:
                        nc.sync.dma_start(
                            out=dmp_den[b * BS:(b + 1) * BS, :], in_=den[:])
                    o1 = wpool.tile([128, Fo], f32, tag="o1")
                    nc.vector.tensor_tensor(
                        out=o1[:].rearrange("p (h f) -> p h f", h=heads),
                        in0=pagg[:].rearrange("p (h f) -> p h f",
                                              h=heads),
                        in1=rec[:].unsqueeze(-1)
                            .broadcast_to([128, heads, HID]),
                        op=Alu.mult)
                    o2 = wpool.tile([128, Fo], f32, tag="o2")
                    nc.vector.tensor_tensor(out=o2[:], in0=o1[:],
                                            in1=brep[:, :Fo], op=Alu.add)
                    if layer == 2:
                        nc.tensor.matmul(out=psum_sum[:],
                                         lhsT=maskc_s[:, b:b + 1],
                                         rhs=o2[:], start=(b == 0),
                                         stop=(b == NB - 1))
                    else:
                        o3 = wpool.tile([128, Fo], f32, tag="o3")
                        nc.scalar.activation(o3[:], o2[:], Act.Relu)
                        if debug and layer == 0:
                            nc.sync.dma_start(
                                out=dmp_h[b * BS:(b + 1) * BS, :], in_=o3[:])
                        for k2 in range(2):
                            pt = ppool1.tile([128, 128], f32, tag="tp",
                                            space="PSUM")
                            nc.tensor.transpose(
                                pt[:], o3[:, k2 * 128:(k2 + 1) * 128],
                                ident_s[:])
                            nc.vector.tensor_copy(
                                out=hT[:, k2, b * BS:(b + 1) * BS],
                                in_=pt[:])
                if layer == 2:
                    osb = spool.tile([1, OUT_C], f32, tag="osb")
                    nc.vector.tensor_copy(out=osb[:], in_=psum_sum[:])
                    nc.sync.dma_start(out=out_d[:], in_=osb[:])

            stage = 0
            for layer in range(3):
                if stage >= upto:
                    break
                ad_all = transform(layer)
                stage += 1
                if stage >= upto:
                    break
                allgather(layer)
                stage += 1
                if stage >= upto:
                    break
                aggregate(layer, ad_all)
                stage += 1

    nc.compile()
    return nc


def _get_built():
    global _BUILT
    if _BUILT is None:
        _BUILT = build_kernel(upto=int(os.environ.get("GAT_UPTO", "99")))
    return _BUILT


def kernel(**inputs) -> np.ndarray:
    from concourse.bass_utils import run_bass_kernel_spmd

    pp = preprocess(np.asarray(inputs["edge_index"]))
    in_maps = build_core_inputs(inputs, pp)
    nc = _get_built()
    res = run_bass_kernel_spmd(nc, in_maps, core_ids=list(range(NCORES)))
    parts = np.stack([r["out_part"][0] for r in res.results])  # [8, 64]
    g = parts.sum(axis=0, keepdims=True) / N
    out = (g @ np.asarray(inputs["hw"], np.float32)
           + np.asarray(inputs["hb"], np.float32)).astype(np.float32)
    return out
